# revision 16
# baseline (speedup 1.0000x reference)
"""Trainium2 Bass kernel for nn_Attention_78898549227880.

Computation (per point p of B*N = 32768 points):
  qe = MLP_q(q[p])                        # 3 -> 64 -> 64 -> 64, ReLU on first two
  ke_j = MLP_k(k[p, j])  j in 0..31
  s_j = (qe . ke_j) / sqrt(3)
  w = softmax(s)
  out[p] = sum_j w_j * v[p, j]            # v rows are 256-wide

Key algebraic restructuring: since softmax is shift invariant,
  s_j = qe . (W2k h2_j + b2k) = (W2k^T qe) . h2_j + const
so the k-MLP's last layer folds into a per-point "effective query"
qt = W2k^T qe (scaled by 1/sqrt(3)); the qe.b2k term cancels in softmax.
Scores are tiny (|s| <= 0.25 for these inputs) so softmax needs no
max-subtraction.

Layouts (per core: P=4096 points, 32 tiles of 128 points, groups of 4
consecutive points, clusters of 8 groups):
  - k-MLP runs with d on partitions: h2 [64, 4096] per tile (bf16).
  - scores per group: stationary lhsT = h2 group block [64, 128],
    moving rhs = qt columns [64, 4] -> psum [128, 4]; the needed scores
    sit at partition (c*32+k), column c.
  - softmax sum over k (which lives on partitions) via a block-ones
    matmul [128,4]^T @ e -> per-point sums; reciprocal; broadcast back
    via a second ones matmul; all in the (c,k)-partition layout.
  - weighted sum as a matmul: stationary block-diagonal W_BLK [128, 4]
    (col c = w values of point c in rows c*32..c*32+31), moving rhs =
    v tile [128 partitions = (c,k), 256] in v's natural DRAM layout.
    Output psum [4, 256] col-tiled 4 groups/bank (partitions 32j+c).

Sharding: data parallel over B*N across 8 cores; weights replicated.
"""

import math
import os
import sys
from contextlib import ExitStack

import numpy as np

for _p in ("/opt/trn_rl_repo", "/root/.axon_site/_ro/trn_rl_repo"):
    if os.path.isdir(_p) and _p not in sys.path:
        sys.path.insert(0, _p)

B, N, K, DV, HID = 4, 8192, 32, 256, 64
PTS = B * N                 # 32768 points
NCORES = 8
P = PTS // NCORES           # 4096 points per core
TILE_PTS = 128              # points per tile
NTILES = P // TILE_PTS      # 32
NGROUPS = TILE_PTS // 4     # 32 groups of 4 points per tile
NCLUST = 4                  # clusters of 8 groups per tile

TRACE = False               # test harness can flip this for profiling
LAST_RESULT = None

_PROGRAM = None


def _build_program():
    import concourse.bass as bass
    import concourse.tile as tile
    from concourse import bacc, mybir

    dt = mybir.dt
    f32, bf16 = dt.float32, dt.bfloat16
    f32r = dt.float32r

    nc = bacc.Bacc("TRN2", target_bir_lowering=False, debug=False)

    # ---- DRAM I/O ----
    kT_d = nc.dram_tensor("kT", [3, P * K], bf16, kind="ExternalInput").ap()
    qT_d = nc.dram_tensor("qT", [3, P], f32r, kind="ExternalInput").ap()
    v_d = nc.dram_tensor("v", [P * K, DV], bf16, kind="ExternalInput").ap()
    kw0T_d = nc.dram_tensor("kw0T", [3, HID], bf16, kind="ExternalInput").ap()
    kw1T_d = nc.dram_tensor("kw1T", [HID, HID], bf16, kind="ExternalInput").ap()
    kw2s_d = nc.dram_tensor("kw2s", [HID, HID], f32r, kind="ExternalInput").ap()
    kb0_d = nc.dram_tensor("kb0", [HID, 1], f32, kind="ExternalInput").ap()
    kb1_d = nc.dram_tensor("kb1", [HID, 1], f32, kind="ExternalInput").ap()
    qw0T_d = nc.dram_tensor("qw0T", [3, HID], f32r, kind="ExternalInput").ap()
    qw1T_d = nc.dram_tensor("qw1T", [HID, HID], f32r, kind="ExternalInput").ap()
    qw2T_d = nc.dram_tensor("qw2T", [HID, HID], f32r, kind="ExternalInput").ap()
    qb0_d = nc.dram_tensor("qb0", [HID, 1], f32, kind="ExternalInput").ap()
    qb1_d = nc.dram_tensor("qb1", [HID, 1], f32, kind="ExternalInput").ap()
    qb2_d = nc.dram_tensor("qb2", [HID, 1], f32, kind="ExternalInput").ap()
    onesblk_d = nc.dram_tensor("onesblk", [128, 4], f32r, kind="ExternalInput").ap()
    zeros_d = nc.dram_tensor("zeros128", [128, 128], bf16, kind="ExternalInput").ap()
    ones4_d = nc.dram_tensor("ones4c", [4, 128], f32r, kind="ExternalInput").ap()
    out_d = nc.dram_tensor("out", [P, DV], f32, kind="ExternalOutput").ap()

    Relu = mybir.ActivationFunctionType.Relu
    Exp = mybir.ActivationFunctionType.Exp
    Identity = mybir.ActivationFunctionType.Identity

    with (
        tile.TileContext(nc) as tc,
        nc.allow_low_precision(reason="f32r matmul inputs are rounded by design"),
        ExitStack() as ctx,
    ):
        const = ctx.enter_context(tc.tile_pool(name="const", bufs=1))
        qphase = ctx.enter_context(tc.tile_pool(name="qphase", bufs=2))
        kin_pool = ctx.enter_context(tc.tile_pool(name="kin", bufs=2))
        h1_pool = ctx.enter_context(tc.tile_pool(name="h1", bufs=2))
        h2_pool = ctx.enter_context(tc.tile_pool(name="h2", bufs=2))
        sm_pool = ctx.enter_context(tc.tile_pool(name="sm", bufs=3))
        v_pool = ctx.enter_context(tc.tile_pool(name="v", bufs=8))
        st_pool = ctx.enter_context(tc.tile_pool(name="st", bufs=3))

        ps_mlp = ctx.enter_context(tc.tile_pool(name="ps_mlp", bufs=2, space="PSUM"))
        ps_sc = ctx.enter_context(tc.tile_pool(name="ps_sc", bufs=2, space="PSUM"))
        ps_s = ctx.enter_context(tc.tile_pool(name="ps_s", bufs=1, space="PSUM"))
        ps_r = ctx.enter_context(tc.tile_pool(name="ps_r", bufs=1, space="PSUM"))
        ps_ws = ctx.enter_context(tc.tile_pool(name="ps_ws", bufs=2, space="PSUM"))

        # ---- constants into SBUF ----
        def load_const(name, dram_ap, shape, dtype):
            t = const.tile(shape, dtype, tag=name, name=name)
            nc.sync.dma_start(t[:], dram_ap)
            return t

        kw0T = load_const("kw0T", kw0T_d, [3, HID], bf16)
        kw1T = load_const("kw1T", kw1T_d, [HID, HID], bf16)
        kw2s = load_const("kw2s", kw2s_d, [HID, HID], f32r)
        kb0 = load_const("kb0", kb0_d, [HID, 1], f32)
        kb1 = load_const("kb1", kb1_d, [HID, 1], f32)
        qw0T = load_const("qw0T", qw0T_d, [3, HID], f32r)
        qw1T = load_const("qw1T", qw1T_d, [HID, HID], f32r)
        qw2T = load_const("qw2T", qw2T_d, [HID, HID], f32r)
        qb0 = load_const("qb0", qb0_d, [HID, 1], f32)
        qb1 = load_const("qb1", qb1_d, [HID, 1], f32)
        qb2 = load_const("qb2", qb2_d, [HID, 1], f32)

        # ones_blk[c*32+k, c] = 1 else 0  (sums e over k per point)
        ones_blk = load_const("ones_blk", onesblk_d, [128, 4], f32r)
        # ones4[c, c*32+k] = 1 else 0  (broadcasts per-point recip back to rows)
        ones4 = load_const("ones4", ones4_d, [4, 128], f32r)

        qt_bf = const.tile([HID, P], bf16, tag="qt_bf")

        # W_BLK double buffer; off-diagonal zeros are written once and
        # never touched again (each tile rewrites the same positions).
        wblk = [
            const.tile([128, 128], bf16, tag=f"wblk{i}", name=f"wblk{i}")
            for i in range(2)
        ]
        for i in range(2):
            nc.sync.dma_start(wblk[i][:], zeros_d)

        # ---- Q phase: qe = MLP_q(q); qt = (k_w2 * scale)^T-fold @ qe ----
        q_in = const.tile([3, P], f32r, tag="q_in")
        nc.sync.dma_start(q_in[:], qT_d)
        NQ = P // 512
        for cc in range(NQ):
            sl = bass.ts(cc, 512)
            ps0 = ps_mlp.tile([HID, 512], f32, tag="ps_mlp")
            nc.tensor.matmul(ps0[:], qw0T[:], q_in[:, sl])
            h1q = qphase.tile([HID, 512], f32r, tag="h1q")
            nc.scalar.activation(h1q[:], ps0[:], Relu, bias=qb0[:])
            ps1 = ps_mlp.tile([HID, 512], f32, tag="ps_mlp")
            nc.tensor.matmul(ps1[:], qw1T[:], h1q[:])
            h2q = qphase.tile([HID, 512], f32r, tag="h2q")
            nc.scalar.activation(h2q[:], ps1[:], Relu, bias=qb1[:])
            ps2 = ps_mlp.tile([HID, 512], f32, tag="ps_mlp")
            nc.tensor.matmul(ps2[:], qw2T[:], h2q[:])
            qe = qphase.tile([HID, 512], f32r, tag="qe")
            nc.scalar.activation(qe[:], ps2[:], Identity, bias=qb2[:])
            ps3 = ps_mlp.tile([HID, 512], f32, tag="ps_mlp")
            nc.tensor.matmul(ps3[:], kw2s[:], qe[:])
            nc.scalar.copy(qt_bf[:, sl], ps3[:])

        # ---- main loop over tiles of 128 points ----
        for t in range(NTILES):
            ptbase = t * TILE_PTS
            rowbase = ptbase * K

            k_in = kin_pool.tile([3, TILE_PTS * K], bf16, tag="k_in")
            nc.sync.dma_start(k_in[:], kT_d[:, rowbase : rowbase + TILE_PTS * K])

            h1 = h1_pool.tile([HID, TILE_PTS * K], bf16, tag="h1")
            h2 = h2_pool.tile([HID, TILE_PTS * K], bf16, tag="h2")
            for cc in range(8):
                sl = bass.ts(cc, 512)
                p0 = ps_mlp.tile([HID, 512], f32, tag="ps_mlp")
                nc.tensor.matmul(p0[:], kw0T[:], k_in[:, sl])
                nc.scalar.activation(h1[:, sl], p0[:], Relu, bias=kb0[:])
                p1 = ps_mlp.tile([HID, 512], f32, tag="ps_mlp")
                nc.tensor.matmul(p1[:], kw1T[:], h1[:, sl])
                nc.scalar.activation(h2[:, sl], p1[:], Relu, bias=kb1[:])

            # scores: group g -> psum cols 4g..4g+3, partitions (c*32+k)
            sc = ps_sc.tile([128, 128], f32, tag="sc")
            for g in range(NGROUPS):
                nc.tensor.matmul(
                    sc[:, 4 * g : 4 * g + 4],
                    h2[:, 128 * g : 128 * g + 128],
                    qt_bf[:, ptbase + 4 * g : ptbase + 4 * g + 4],
                )

            # softmax without max-subtraction (scores are tiny)
            e_t = sm_pool.tile([128, NGROUPS], f32r, tag="e_t")
            for c in range(4):
                nc.scalar.activation(
                    e_t[32 * c : 32 * c + 32, :],
                    sc[32 * c : 32 * c + 32, c::4],
                    Exp,
                )
            s_ps = ps_s.tile([4, NGROUPS], f32, tag="s_ps")
            nc.tensor.matmul(s_ps[:], ones_blk[:], e_t[:])
            r_sb = sm_pool.tile([4, NGROUPS], f32r, tag="r_sb")
            nc.vector.reciprocal(r_sb[:], s_ps[:])
            r_ps = ps_r.tile([128, NGROUPS], f32, tag="r_ps")
            nc.tensor.matmul(r_ps[:], ones4[:], r_sb[:])
            w_sb = sm_pool.tile([128, NGROUPS], f32, tag="w_sb")
            nc.vector.tensor_mul(w_sb[:], e_t[:], r_ps[:])

            # scatter w into block-diagonal stationary
            wb = wblk[t % 2]
            for c in range(4):
                nc.vector.tensor_copy(
                    wb[32 * c : 32 * c + 32, c::4],
                    w_sb[32 * c : 32 * c + 32, :],
                )

            # weighted sum over v, one cluster = 8 groups = 32 points
            for cl in range(NCLUST):
                rs = rowbase + cl * 1024
                v_sb = v_pool.tile([128, 8 * DV], bf16, tag="v_sb")
                nc.sync.dma_start(
                    v_sb[:].rearrange("p (g d) -> p g d", g=8),
                    v_d[rs : rs + 1024, :].rearrange("(g ck) d -> ck g d", g=8),
                )
                ws = ps_ws.tile([128, 512], f32, tag="ws")
                for gi in range(8):
                    g = 8 * cl + gi
                    j, h = gi % 4, gi // 4
                    nc.tensor.matmul(
                        ws[32 * j : 32 * j + 4, 256 * h : 256 * h + 256],
                        wb[:, 4 * g : 4 * g + 4],
                        v_sb[:, 256 * gi : 256 * gi + 256],
                        tile_position=(0, 32 * j),
                    )
                st = st_pool.tile([128, 512], f32, tag="st")
                nc.vector.tensor_copy(st[:], ws[:])
                # partition 32j+c, col half h holds point (32cl + 16h + 4j + c)
                ob = ptbase + 32 * cl
                src = st[:].rearrange("(j cc) (h d) -> j cc h d", j=4, h=2)
                dst = out_d[ob : ob + 32, :].rearrange("(h j c) d -> j c h d", h=2, j=4)
                for j in range(4):
                    nc.sync.dma_start(dst[j], src[j, 0:4])

    nc.compile()
    return nc


def _get_program():
    global _PROGRAM
    if _PROGRAM is None:
        _PROGRAM = _build_program()
    return _PROGRAM


def _prepare(inputs):
    """Build (or reuse) the Bass program and per-core input maps."""
    import ml_dtypes

    bf16 = ml_dtypes.bfloat16
    f32 = np.float32
    scale = 1.0 / math.sqrt(3.0)

    q = np.asarray(inputs["q"], dtype=f32).reshape(PTS, 3)
    k = np.asarray(inputs["k"], dtype=f32).reshape(PTS * K, 3)
    v_bf = np.asarray(inputs["v"], dtype=f32).reshape(PTS * K, DV).astype(bf16)

    shared = {
        "kw0T": np.ascontiguousarray(np.asarray(inputs["k_w0"], f32).T.astype(bf16)),
        "kw1T": np.ascontiguousarray(np.asarray(inputs["k_w1"], f32).T.astype(bf16)),
        "kw2s": np.ascontiguousarray(np.asarray(inputs["k_w2"], f32) * scale),
        "kb0": np.ascontiguousarray(np.asarray(inputs["k_b0"], f32).reshape(HID, 1)),
        "kb1": np.ascontiguousarray(np.asarray(inputs["k_b1"], f32).reshape(HID, 1)),
        "qw0T": np.ascontiguousarray(np.asarray(inputs["q_w0"], f32).T),
        "qw1T": np.ascontiguousarray(np.asarray(inputs["q_w1"], f32).T),
        "qw2T": np.ascontiguousarray(np.asarray(inputs["q_w2"], f32).T),
        "qb0": np.ascontiguousarray(np.asarray(inputs["q_b0"], f32).reshape(HID, 1)),
        "qb1": np.ascontiguousarray(np.asarray(inputs["q_b1"], f32).reshape(HID, 1)),
        "qb2": np.ascontiguousarray(np.asarray(inputs["q_b2"], f32).reshape(HID, 1)),
    }
    ones_blk = np.zeros((128, 4), f32)
    ones4 = np.zeros((4, 128), f32)
    for c in range(4):
        ones_blk[32 * c : 32 * c + 32, c] = 1.0
        ones4[c, 32 * c : 32 * c + 32] = 1.0
    shared["onesblk"] = ones_blk
    shared["ones4c"] = ones4
    shared["zeros128"] = np.zeros((128, 128), bf16)

    in_maps = []
    for core in range(NCORES):
        a, b = core * P, (core + 1) * P
        m = dict(shared)
        m["kT"] = np.ascontiguousarray(k[a * K : b * K].astype(bf16).T)
        m["qT"] = np.ascontiguousarray(q[a:b].T)
        m["v"] = v_bf[a * K : b * K]
        in_maps.append(m)

    return _get_program(), in_maps


def kernel(**inputs):
    global LAST_RESULT
    from concourse.bass_utils import run_bass_kernel_spmd

    nc, in_maps = _prepare(inputs)
    res = run_bass_kernel_spmd(nc, in_maps, list(range(NCORES)), trace=TRACE)
    LAST_RESULT = res
    out = np.concatenate([res.results[i]["out"] for i in range(NCORES)], axis=0)
    return np.ascontiguousarray(out.reshape(B, N, DV).astype(np.float32))


# revision 17
# speedup vs baseline: 136.9475x; 136.9475x over previous
"""Trainium2 Bass kernel for nn_Attention_78898549227880.

Computation (per point p of B*N = 32768 points):
  qe = MLP_q(q[p])                        # 3 -> 64 -> 64 -> 64, ReLU on first two
  ke_j = MLP_k(k[p, j])  j in 0..31
  s_j = (qe . ke_j) / sqrt(3)
  w = softmax(s)
  out[p] = sum_j w_j * v[p, j]            # v rows are 256-wide

Key algebraic restructuring: since softmax is shift invariant,
  s_j = qe . (W2k h2_j + b2k) = (W2k^T qe) . h2_j + const
so the k-MLP's last layer folds into a per-point "effective query"
qt = W2k^T qe (scaled by 1/sqrt(3)); the qe.b2k term cancels in softmax.
Scores are tiny (|s| <= 0.25 for these inputs) so softmax needs no
max-subtraction.

Layouts (per core: P=4096 points, 32 tiles of 128 points, groups of 4
consecutive points, clusters of 8 groups):
  - k-MLP runs with d on partitions: h2 [64, 4096] per tile (bf16).
  - scores per group: stationary lhsT = h2 group block [64, 128],
    moving rhs = qt columns [64, 4] -> psum [128, 4]; the needed scores
    sit at partition (c*32+k), column c.
  - softmax sum over k (which lives on partitions) via a block-ones
    matmul [128,4]^T @ e -> per-point sums; reciprocal; broadcast back
    via a second ones matmul; all in the (c,k)-partition layout.
  - weighted sum as a matmul: stationary block-diagonal W_BLK [128, 4]
    (col c = w values of point c in rows c*32..c*32+31), moving rhs =
    v tile [128 partitions = (c,k), 256] in v's natural DRAM layout.
    Output psum [4, 256] col-tiled 4 groups/bank (partitions 32j+c).

Sharding: data parallel over B*N across 8 cores; weights replicated.
"""

import math
import os
import sys
from contextlib import ExitStack

import numpy as np

for _p in ("/opt/trn_rl_repo", "/root/.axon_site/_ro/trn_rl_repo"):
    if os.path.isdir(_p) and _p not in sys.path:
        sys.path.insert(0, _p)

B, N, K, DV, HID = 4, 8192, 32, 256, 64
PTS = B * N                 # 32768 points
NCORES = 8
P = PTS // NCORES           # 4096 points per core
TILE_PTS = 128              # points per tile
NTILES = P // TILE_PTS      # 32
NGROUPS = TILE_PTS // 4     # 32 groups of 4 points per tile
NCLUST = 4                  # clusters of 8 groups per tile

TRACE = False               # test harness can flip this for profiling
LAST_RESULT = None

_PROGRAM = {}


def _build_program(repeat=1):
    import concourse.bass as bass
    import concourse.tile as tile
    from concourse import bacc, mybir

    dt = mybir.dt
    f32, bf16 = dt.float32, dt.bfloat16
    f32r = dt.float32r

    nc = bacc.Bacc("TRN2", target_bir_lowering=False, debug=False)

    # ---- DRAM I/O ----
    kT_d = nc.dram_tensor("kT", [3, P * K], bf16, kind="ExternalInput").ap()
    qT_d = nc.dram_tensor("qT", [3, P], f32r, kind="ExternalInput").ap()
    v_d = nc.dram_tensor("v", [P * K, DV], bf16, kind="ExternalInput").ap()
    kw0T_d = nc.dram_tensor("kw0T", [3, HID], bf16, kind="ExternalInput").ap()
    kw1T_d = nc.dram_tensor("kw1T", [HID, HID], bf16, kind="ExternalInput").ap()
    kw2s_d = nc.dram_tensor("kw2s", [HID, HID], f32r, kind="ExternalInput").ap()
    kb0_d = nc.dram_tensor("kb0", [HID, 1], f32, kind="ExternalInput").ap()
    kb1_d = nc.dram_tensor("kb1", [HID, 1], f32, kind="ExternalInput").ap()
    qw0T_d = nc.dram_tensor("qw0T", [3, HID], f32r, kind="ExternalInput").ap()
    qw1T_d = nc.dram_tensor("qw1T", [HID, HID], f32r, kind="ExternalInput").ap()
    qw2T_d = nc.dram_tensor("qw2T", [HID, HID], f32r, kind="ExternalInput").ap()
    qb0_d = nc.dram_tensor("qb0", [HID, 1], f32, kind="ExternalInput").ap()
    qb1_d = nc.dram_tensor("qb1", [HID, 1], f32, kind="ExternalInput").ap()
    qb2_d = nc.dram_tensor("qb2", [HID, 1], f32, kind="ExternalInput").ap()
    onesblk_d = nc.dram_tensor("onesblk", [128, 4], f32r, kind="ExternalInput").ap()
    zeros_d = nc.dram_tensor("zeros128", [128, 128], bf16, kind="ExternalInput").ap()
    ones4_d = nc.dram_tensor("ones4c", [4, 128], f32r, kind="ExternalInput").ap()
    out_d = nc.dram_tensor("out", [P, DV], f32, kind="ExternalOutput").ap()

    Relu = mybir.ActivationFunctionType.Relu
    Exp = mybir.ActivationFunctionType.Exp
    Identity = mybir.ActivationFunctionType.Identity

    with (
        tile.TileContext(nc) as tc,
        nc.allow_low_precision(reason="f32r matmul inputs are rounded by design"),
        ExitStack() as ctx,
    ):
        const = ctx.enter_context(tc.tile_pool(name="const", bufs=1))
        qphase = ctx.enter_context(tc.tile_pool(name="qphase", bufs=2))
        kin_pool = ctx.enter_context(tc.tile_pool(name="kin", bufs=2))
        h1_pool = ctx.enter_context(tc.tile_pool(name="h1", bufs=2))
        h2_pool = ctx.enter_context(tc.tile_pool(name="h2", bufs=2))
        sm_pool = ctx.enter_context(tc.tile_pool(name="sm", bufs=3))
        v_pool = ctx.enter_context(tc.tile_pool(name="v", bufs=8))
        st_pool = ctx.enter_context(tc.tile_pool(name="st", bufs=3))

        ps_mlp = ctx.enter_context(tc.tile_pool(name="ps_mlp", bufs=2, space="PSUM"))
        ps_sc = ctx.enter_context(tc.tile_pool(name="ps_sc", bufs=2, space="PSUM"))
        ps_s = ctx.enter_context(tc.tile_pool(name="ps_s", bufs=1, space="PSUM"))
        ps_r = ctx.enter_context(tc.tile_pool(name="ps_r", bufs=1, space="PSUM"))
        ps_ws = ctx.enter_context(tc.tile_pool(name="ps_ws", bufs=2, space="PSUM"))

        # ---- constants into SBUF ----
        def load_const(name, dram_ap, shape, dtype):
            t = const.tile(shape, dtype, tag=name, name=name)
            nc.sync.dma_start(t[:], dram_ap)
            return t

        kw0T = load_const("kw0T", kw0T_d, [3, HID], bf16)
        kw1T = load_const("kw1T", kw1T_d, [HID, HID], bf16)
        kw2s = load_const("kw2s", kw2s_d, [HID, HID], f32r)
        kb0 = load_const("kb0", kb0_d, [HID, 1], f32)
        kb1 = load_const("kb1", kb1_d, [HID, 1], f32)
        qw0T = load_const("qw0T", qw0T_d, [3, HID], f32r)
        qw1T = load_const("qw1T", qw1T_d, [HID, HID], f32r)
        qw2T = load_const("qw2T", qw2T_d, [HID, HID], f32r)
        qb0 = load_const("qb0", qb0_d, [HID, 1], f32)
        qb1 = load_const("qb1", qb1_d, [HID, 1], f32)
        qb2 = load_const("qb2", qb2_d, [HID, 1], f32)

        # ones_blk[c*32+k, c] = 1 else 0  (sums e over k per point)
        ones_blk = load_const("ones_blk", onesblk_d, [128, 4], f32r)
        # ones4[c, c*32+k] = 1 else 0  (broadcasts per-point recip back to rows)
        ones4 = load_const("ones4", ones4_d, [4, 128], f32r)

        qt_bf = const.tile([HID, P], bf16, tag="qt_bf")

        # W_BLK double buffer; off-diagonal zeros are written once and
        # never touched again (each tile rewrites the same positions).
        wblk = [
            const.tile([128, 128], bf16, tag=f"wblk{i}", name=f"wblk{i}")
            for i in range(2)
        ]
        for i in range(2):
            nc.sync.dma_start(wblk[i][:], zeros_d)

        # ---- Q phase: qe = MLP_q(q); qt = (k_w2 * scale)^T-fold @ qe ----
        q_in = const.tile([3, P], f32r, tag="q_in")
        nc.sync.dma_start(q_in[:], qT_d)
        NQ = P // 512
        for cc in range(NQ):
            sl = bass.ts(cc, 512)
            ps0 = ps_mlp.tile([HID, 512], f32, tag="ps_mlp")
            nc.tensor.matmul(ps0[:], qw0T[:], q_in[:, sl])
            h1q = qphase.tile([HID, 512], f32r, tag="h1q")
            nc.scalar.activation(h1q[:], ps0[:], Relu, bias=qb0[:])
            ps1 = ps_mlp.tile([HID, 512], f32, tag="ps_mlp")
            nc.tensor.matmul(ps1[:], qw1T[:], h1q[:])
            h2q = qphase.tile([HID, 512], f32r, tag="h2q")
            nc.scalar.activation(h2q[:], ps1[:], Relu, bias=qb1[:])
            ps2 = ps_mlp.tile([HID, 512], f32, tag="ps_mlp")
            nc.tensor.matmul(ps2[:], qw2T[:], h2q[:])
            qe = qphase.tile([HID, 512], f32r, tag="qe")
            nc.scalar.activation(qe[:], ps2[:], Identity, bias=qb2[:])
            ps3 = ps_mlp.tile([HID, 512], f32, tag="ps_mlp")
            nc.tensor.matmul(ps3[:], kw2s[:], qe[:])
            nc.scalar.copy(qt_bf[:, sl], ps3[:])

        # ---- main loop over tiles of 128 points ----
        for t in range(NTILES * repeat):
            t = t % NTILES
            ptbase = t * TILE_PTS
            rowbase = ptbase * K

            k_in = kin_pool.tile([3, TILE_PTS * K], bf16, tag="k_in")
            nc.sync.dma_start(k_in[:], kT_d[:, rowbase : rowbase + TILE_PTS * K])

            h1 = h1_pool.tile([HID, TILE_PTS * K], bf16, tag="h1")
            h2 = h2_pool.tile([HID, TILE_PTS * K], bf16, tag="h2")
            for cc in range(8):
                sl = bass.ts(cc, 512)
                p0 = ps_mlp.tile([HID, 512], f32, tag="ps_mlp")
                nc.tensor.matmul(p0[:], kw0T[:], k_in[:, sl])
                nc.scalar.activation(h1[:, sl], p0[:], Relu, bias=kb0[:])
                p1 = ps_mlp.tile([HID, 512], f32, tag="ps_mlp")
                nc.tensor.matmul(p1[:], kw1T[:], h1[:, sl])
                nc.scalar.activation(h2[:, sl], p1[:], Relu, bias=kb1[:])

            # scores: group g -> psum cols 4g..4g+3, partitions (c*32+k)
            sc = ps_sc.tile([128, 128], f32, tag="sc")
            for g in range(NGROUPS):
                nc.tensor.matmul(
                    sc[:, 4 * g : 4 * g + 4],
                    h2[:, 128 * g : 128 * g + 128],
                    qt_bf[:, ptbase + 4 * g : ptbase + 4 * g + 4],
                )

            # softmax without max-subtraction (scores are tiny)
            e_t = sm_pool.tile([128, NGROUPS], f32r, tag="e_t")
            for c in range(4):
                nc.scalar.activation(
                    e_t[32 * c : 32 * c + 32, :],
                    sc[32 * c : 32 * c + 32, c::4],
                    Exp,
                )
            s_ps = ps_s.tile([4, NGROUPS], f32, tag="s_ps")
            nc.tensor.matmul(s_ps[:], ones_blk[:], e_t[:])
            r_sb = sm_pool.tile([4, NGROUPS], f32r, tag="r_sb")
            nc.vector.reciprocal(r_sb[:], s_ps[:])
            r_ps = ps_r.tile([128, NGROUPS], f32, tag="r_ps")
            nc.tensor.matmul(r_ps[:], ones4[:], r_sb[:])
            w_sb = sm_pool.tile([128, NGROUPS], f32, tag="w_sb")
            nc.vector.tensor_mul(w_sb[:], e_t[:], r_ps[:])

            # scatter w into block-diagonal stationary
            wb = wblk[t % 2]
            for c in range(4):
                nc.vector.tensor_copy(
                    wb[32 * c : 32 * c + 32, c::4],
                    w_sb[32 * c : 32 * c + 32, :],
                )

            # weighted sum over v, one cluster = 8 groups = 32 points
            for cl in range(NCLUST):
                rs = rowbase + cl * 1024
                v_sb = v_pool.tile([128, 8 * DV], bf16, tag="v_sb")
                nc.sync.dma_start(
                    v_sb[:].rearrange("p (g d) -> p g d", g=8),
                    v_d[rs : rs + 1024, :].rearrange("(g ck) d -> ck g d", g=8),
                )
                ws = ps_ws.tile([128, 512], f32, tag="ws")
                for gi in range(8):
                    g = 8 * cl + gi
                    j, h = gi % 4, gi // 4
                    nc.tensor.matmul(
                        ws[32 * j : 32 * j + 4, 256 * h : 256 * h + 256],
                        wb[:, 4 * g : 4 * g + 4],
                        v_sb[:, 256 * gi : 256 * gi + 256],
                        tile_position=(0, 32 * j),
                    )
                st = st_pool.tile([128, 512], f32, tag="st")
                nc.vector.tensor_copy(st[:], ws[:])
                # partition 32j+c, col half h holds point (32cl + 16h + 4j + c)
                ob = ptbase + 32 * cl
                src = st[:].rearrange("(j cc) (h d) -> j cc h d", j=4, h=2)
                dst = out_d[ob : ob + 32, :].rearrange("(h j c) d -> j c h d", h=2, j=4)
                for j in range(4):
                    nc.sync.dma_start(dst[j], src[j, 0:4])

    nc.compile()
    return nc


def _get_program(repeat=1):
    global _PROGRAM
    if _PROGRAM is None:
        _PROGRAM = {}
    if repeat not in _PROGRAM:
        _PROGRAM[repeat] = _build_program(repeat)
    return _PROGRAM[repeat]


def _prepare(inputs):
    """Build (or reuse) the Bass program and per-core input maps."""
    import ml_dtypes

    bf16 = ml_dtypes.bfloat16
    f32 = np.float32
    scale = 1.0 / math.sqrt(3.0)

    q = np.asarray(inputs["q"], dtype=f32).reshape(PTS, 3)
    k = np.asarray(inputs["k"], dtype=f32).reshape(PTS * K, 3)
    v_bf = np.asarray(inputs["v"], dtype=f32).reshape(PTS * K, DV).astype(bf16)

    shared = {
        "kw0T": np.ascontiguousarray(np.asarray(inputs["k_w0"], f32).T.astype(bf16)),
        "kw1T": np.ascontiguousarray(np.asarray(inputs["k_w1"], f32).T.astype(bf16)),
        "kw2s": np.ascontiguousarray(np.asarray(inputs["k_w2"], f32) * scale),
        "kb0": np.ascontiguousarray(np.asarray(inputs["k_b0"], f32).reshape(HID, 1)),
        "kb1": np.ascontiguousarray(np.asarray(inputs["k_b1"], f32).reshape(HID, 1)),
        "qw0T": np.ascontiguousarray(np.asarray(inputs["q_w0"], f32).T),
        "qw1T": np.ascontiguousarray(np.asarray(inputs["q_w1"], f32).T),
        "qw2T": np.ascontiguousarray(np.asarray(inputs["q_w2"], f32).T),
        "qb0": np.ascontiguousarray(np.asarray(inputs["q_b0"], f32).reshape(HID, 1)),
        "qb1": np.ascontiguousarray(np.asarray(inputs["q_b1"], f32).reshape(HID, 1)),
        "qb2": np.ascontiguousarray(np.asarray(inputs["q_b2"], f32).reshape(HID, 1)),
    }
    ones_blk = np.zeros((128, 4), f32)
    ones4 = np.zeros((4, 128), f32)
    for c in range(4):
        ones_blk[32 * c : 32 * c + 32, c] = 1.0
        ones4[c, 32 * c : 32 * c + 32] = 1.0
    shared["onesblk"] = ones_blk
    shared["ones4c"] = ones4
    shared["zeros128"] = np.zeros((128, 128), bf16)

    in_maps = []
    for core in range(NCORES):
        a, b = core * P, (core + 1) * P
        m = dict(shared)
        m["kT"] = np.ascontiguousarray(k[a * K : b * K].astype(bf16).T)
        m["qT"] = np.ascontiguousarray(q[a:b].T)
        m["v"] = v_bf[a * K : b * K]
        in_maps.append(m)

    return _get_program(), in_maps


def kernel(**inputs):
    global LAST_RESULT
    from concourse.bass_utils import run_bass_kernel_spmd

    nc, in_maps = _prepare(inputs)
    res = run_bass_kernel_spmd(nc, in_maps, list(range(NCORES)), trace=TRACE)
    LAST_RESULT = res
    out = np.concatenate([res.results[i]["out"] for i in range(NCORES)], axis=0)
    return np.ascontiguousarray(out.reshape(B, N, DV).astype(np.float32))
